# revision 3
# baseline (speedup 1.0000x reference)
"""Single-head causal attention (B=8, T=2048, D=512, H=64) on 8 TRN2 cores.

Data-parallel: one batch element per NeuronCore. Each core computes
attention in the S^T layout (keys on partitions, queries on the free axis):

  qT/kT/vT [64, T] = W.T @ x.T        (fp16 matmuls, c-tile chunks)
  v        [T, 64] via PE transpose of vT, with a ones column appended
  S^T[j,i] = kT_jblock.T @ qT          (strips of causal width)
  P^T      = exp(S^T / 8)              (ScalarE; no max-subtraction:
                                        scores bounded well below fp16 max)
  out^T[h,i], l[i] = [v|1]_jb.T @ P^T  (accumulated over j-blocks in PSUM;
                                        row 64 is the softmax denominator)

The kernel returns the unnormalized [65, T] strip per core; the host
divides by the denominator row and transposes back to [T, 64].

x is shipped as four tc-major planes so every DMA descriptor moves a
contiguous 4KB row per partition; weights are shipped first so the
first projection can start as soon as the first x plane lands.
"""

import sys

sys.path.insert(0, "/opt/trn_rl_repo")

import numpy as np

import concourse.bass as bass
import concourse.mybir as mybir
import concourse.tile as tile

B, T, D, H = 8, 2048, 512, 64
N_CORES = 8
HALF = T // 2  # i-axis pass width

f32 = mybir.dt.float32
f16 = mybir.dt.float16

_cache = {}


def _legalize_waits(nc, max_waits=1):
    """Walrus codegen accepts at most one sync wait per instruction; hoist
    extras onto same-engine NOPs placed immediately before (engine queues
    are FIFO so blocking semantics are unchanged)."""
    counter = 0
    for bb in nc.main_func.blocks:
        if not any(
            ins.sync_info is not None and len(ins.sync_info.on_wait) > max_waits
            for ins in bb.instructions
        ):
            continue
        new_list = []
        for ins in bb.instructions:
            si = ins.sync_info
            if si is not None and len(si.on_wait) > max_waits:
                waits = list(si.on_wait)
                hoist, keep = waits[:-max_waits], waits[-max_waits:]
                for w in hoist:
                    counter += 1
                    new_list.append(
                        mybir.InstNoOp(
                            name=f"I-waitfix-{counter}",
                            engine=ins.engine,
                            sync_info=mybir.SyncInfo(on_wait=[w], on_update=[]),
                            bass_nofuse=True,
                        )
                    )
                ins.sync_info = mybir.SyncInfo(
                    on_wait=keep, on_update=list(si.on_update)
                )
            new_list.append(ins)
        bb.instructions = new_list
    return counter


def _chunks(lo, hi, step, align):
    """Split [lo, hi) at multiples of `step` relative to `align`."""
    out = []
    cur = lo
    while cur < hi:
        nxt = min(hi, align + ((cur - align) // step + 1) * step)
        out.append((cur, nxt))
        cur = nxt
    return out


def _build():
    nc = bass.Bass()

    # x.T in four tc-major planes: plane tc row p holds
    # [x.T[128c+p, 512tc:512(tc+1)] for c in 0..3] contiguously (4KB).
    xt_d = [
        nc.declare_dram_parameter(f"xt{tc}", [128, 4 * 512], f16, isOutput=False)
        for tc in range(4)
    ]
    # weights first (gate the first matmul), remaining consts second
    cw_d = nc.declare_dram_parameter("cw", [128, 512], f16, isOutput=False)
    # [wv c0..c3 (256) | mask (128) | ones (16) | ident (64)]
    crest_d = nc.declare_dram_parameter("crest", [128, 464], f16, isOutput=False)
    out_d = nc.declare_dram_parameter("out", [H + 1, T], f32, isOutput=True)

    NC_TILES = D // 128  # 4 c-tiles

    with tile.TileContext(nc) as tc_ctx:
        with (
            tc_ctx.tile_pool(name="const", bufs=1) as cpool,
            tc_ctx.tile_pool(name="xt", bufs=1) as xpool,
            tc_ctx.tile_pool(name="qkv", bufs=1) as qkvpool,
            tc_ctx.tile_pool(name="p", bufs=2) as ppool,
            tc_ctx.tile_pool(name="o", bufs=2) as opool,
            tc_ctx.tile_pool(name="ps_proj", bufs=2, space="PSUM") as ps_proj,
            tc_ctx.tile_pool(name="ps_s", bufs=2, space="PSUM") as ps_s,
            tc_ctx.tile_pool(name="ps_pv", bufs=1, space="PSUM") as ps_pv,
        ):
            cw = cpool.tile([128, 512], f16)
            crest = cpool.tile([128, 464], f16)
            xhi_all = xpool.tile([128, 4, 4, 512], f16)

            # issue all input DMAs up front, split across two engines so
            # the issue cost (~0.7us each) overlaps
            nc.sync.dma_start(cw[:], cw_d[:])
            nc.sync.dma_start(xhi_all[:, 0, :, :], xt_d[0].rearrange("p (c t) -> p c t", c=4))
            nc.sync.dma_start(xhi_all[:, 1, :, :], xt_d[1].rearrange("p (c t) -> p c t", c=4))
            nc.gpsimd.dma_start(crest[:], crest_d[:])
            nc.gpsimd.dma_start(xhi_all[:, 2, :, :], xt_d[2].rearrange("p (c t) -> p c t", c=4))
            nc.gpsimd.dma_start(xhi_all[:, 3, :, :], xt_d[3].rearrange("p (c t) -> p c t", c=4))

            wqk = [cw[:, 128 * c : 128 * (c + 1)] for c in range(NC_TILES)]
            wv = [crest[:, 64 * c : 64 * (c + 1)] for c in range(NC_TILES)]
            mask16 = crest[:, 256:384]
            ones = crest[:, 384:400]
            ident16 = crest[0:H, 400:464]

            # HAM warm-up: dense bf16 matmuls while the input DMAs run so
            # the clock ramp engages before real work starts; also touch
            # Exp so the ACT table set loads now (1.3us) instead of on the
            # first real strip.
            warm_bf = cpool.tile([128, 512], mybir.dt.bfloat16)
            nc.vector.memset(warm_bf[:], 1.0)
            exp_warm = cpool.tile([1, 2], f32)
            nc.scalar.activation(
                exp_warm[:], warm_bf[0:1, 0:2], mybir.ActivationFunctionType.Exp
            )
            warm_ps = ps_s.tile([128, 512], f32, tag="s", name="warm_ps")
            for _ in range(8):
                nc.tensor.matmul(
                    warm_ps[:], warm_bf[:, 0:128], warm_bf[:], start=True, stop=True
                )

            qT = qkvpool.tile([H, T], f16)
            kT = qkvpool.tile([H, T], f16)
            vT = qkvpool.tile([H, T], f16)
            v1 = qkvpool.tile([128, T // 128, H + 1], f16)
            nc.vector.tensor_copy(v1[:, :, H : H + 1], ones)

            def x_ap(tc512, c):
                return xhi_all[:, tc512 // 512, c, :]

            def proj_qk_unit(tc512):
                qk_ps = ps_proj.tile([128, 512], f32, tag="work", name="qk_ps")
                for c in range(NC_TILES):
                    nc.tensor.matmul(
                        qk_ps[:],
                        wqk[c],
                        x_ap(tc512, c),
                        start=(c == 0),
                        stop=(c == NC_TILES - 1),
                    )
                nc.vector.tensor_copy(qT[:, tc512 : tc512 + 512], qk_ps[0:H, :])
                nc.vector.tensor_copy(kT[:, tc512 : tc512 + 512], qk_ps[H : 2 * H, :])

            def proj_v_unit(tc512):
                v_ps = ps_proj.tile([128, 512], f32, tag="work", name="v_ps")
                for c in range(NC_TILES):
                    nc.tensor.matmul(
                        v_ps[0:H, :],
                        wv[c],
                        x_ap(tc512, c),
                        start=(c == 0),
                        stop=(c == NC_TILES - 1),
                    )
                nc.vector.tensor_copy(vT[:, tc512 : tc512 + 512], v_ps[0:H, :])

            def vtrans_unit(jj_pair):
                vt_ps = ps_proj.tile([128, 2, H], f16, tag="work", name="vt_ps")
                for jl, jj in enumerate(jj_pair):
                    nc.tensor.transpose(
                        vt_ps[:, jl, :],
                        vT[:, 128 * jj : 128 * (jj + 1)],
                        ident16,
                    )
                    nc.vector.tensor_copy(v1[:, jj, 0:H], vt_ps[:, jl, :])

            def attn_S(t0, jb, lim=None):
                # S^T strip matmuls for one j-block, cols [i_start, t0+lim)
                i_start = max(t0, 128 * jb)
                W = t0 + (lim if lim is not None else HALF) - i_start
                s_ps = ps_s.tile([128, HALF], f32, tag="s", name="s_ps")
                for ls, le in _chunks(0, W, 512, 0):
                    nc.tensor.matmul(
                        s_ps[:, ls:le],
                        kT[:, 128 * jb : 128 * (jb + 1)],
                        qT[:, i_start + ls : i_start + le],
                        start=True,
                        stop=True,
                    )
                return s_ps

            def attn_S_more(t0, jb, s_ps, lo_lim, hi_lim):
                # extend an existing strip: cols [t0+lo_lim, t0+hi_lim)
                i_start = max(t0, 128 * jb)
                for ls, le in _chunks(t0 + lo_lim - i_start, t0 + hi_lim - i_start, 512, 0):
                    nc.tensor.matmul(
                        s_ps[:, ls:le],
                        kT[:, 128 * jb : 128 * (jb + 1)],
                        qT[:, i_start + ls : i_start + le],
                        start=True,
                        stop=True,
                    )

            def attn_exp(t0, jb, s_ps, lo=0, hi=None, p_sb=None, mask=True):
                i_start = max(t0, 128 * jb)
                W = t0 + HALF - i_start
                if hi is None:
                    hi = W
                if p_sb is None:
                    p_sb = ppool.tile(
                        [128, HALF], f16, tag="p", name="p_sb", bufs=4
                    )
                nc.scalar.activation(
                    p_sb[:, lo:hi],
                    s_ps[:, lo:hi],
                    mybir.ActivationFunctionType.Exp,
                    scale=1.0 / 8.0,
                )
                if mask and 128 * jb >= t0 and lo == 0:
                    nc.vector.tensor_mul(p_sb[:, 0:128], p_sb[:, 0:128], mask16)
                return p_sb

            def exp_store(t0, jb, s_ps):
                # exp into a held P slot (pass-1 strips precomputed during
                # pass-0; no mask: these are non-diagonal)
                W = t0 + HALF - max(t0, 128 * jb)
                p_sb = ppool.tile([128, HALF], f16, tag="ppre", name="p_pre", bufs=6)
                nc.scalar.activation(
                    p_sb[:, 0:W],
                    s_ps[:, 0:W],
                    mybir.ActivationFunctionType.Exp,
                    scale=1.0 / 8.0,
                )
                return p_sb

            def attn_pv(t0, n_jb, pv_ps, jb, p_sb):
                i_start = max(t0, 128 * jb)
                for gs, ge in _chunks(i_start, t0 + HALF, 512, 0):
                    ic_last_jb = min(n_jb - 1, (ge - 1) // 128)
                    nc.tensor.matmul(
                        pv_ps[:, gs - t0 : ge - t0],
                        v1[:, jb, :],
                        p_sb[:, gs - i_start : ge - i_start],
                        start=(jb == 0),
                        stop=(jb == ic_last_jb),
                    )

            def out_chunk(pv_ps, t0, c):
                lo, hi = 512 * c, 512 * (c + 1)
                out_sb = opool.tile([H + 1, 512], f32, tag="o", name="out_sb")
                nc.vector.tensor_copy(out_sb[:, 0 : hi - lo], pv_ps[:, lo:hi])
                nc.sync.dma_start(
                    out_d[:, t0 + lo : t0 + hi], out_sb[:, 0 : hi - lo]
                )

            # --- startup: get the first exp going as early as possible ---
            proj_qk_unit(0)
            s00 = attn_S(0, 0, lim=512)  # S(0,0) cols [0:512)
            p00 = attn_exp(0, 0, s00, lo=0, hi=512)
            proj_v_unit(0)
            proj_qk_unit(512)
            attn_S_more(0, 0, s00, 512, 1024)
            attn_exp(0, 0, s00, lo=512, hi=1024, p_sb=p00, mask=False)
            vtrans_unit((0, 1))

            # --- attention pass 0 (i in [0,1024)), units woven in to keep
            # PE fed while ACT drains the strips ---
            weave = {
                1: [lambda: vtrans_unit((2, 3)), lambda: proj_qk_unit(1024)],
                2: [lambda: proj_v_unit(512)],
                3: [lambda: vtrans_unit((4, 5)), lambda: proj_qk_unit(1536)],
                4: [lambda: vtrans_unit((6, 7))],
                6: [lambda: proj_v_unit(1024)],
            }
            pv_ps0 = ps_pv.tile([H + 1, HALF], f32, tag="pv", name="pv_ps")
            p_cur = p00
            pre_p = []
            npre = 6
            for jb in range(8):
                s_nxt = attn_S(0, jb + 1) if jb + 1 < 8 else None
                attn_pv(0, 8, pv_ps0, jb, p_cur)
                if s_nxt is not None:
                    p_cur = attn_exp(0, jb + 1, s_nxt)
                if jb == 3:
                    out_chunk(pv_ps0, 0, 0)
                for u in weave.get(jb, ()):
                    u()
                if jb >= 4:
                    # PE slack late in pass 0: precompute pass-1 strips
                    jbp = len(pre_p)
                    s_pre = attn_S(HALF, jbp)
                    pre_p.append(exp_store(HALF, jbp, s_pre))
            proj_v_unit(1536)
            vtrans_unit((8, 9))
            while len(pre_p) < npre:
                jbp = len(pre_p)
                s_pre = attn_S(HALF, jbp)
                pre_p.append(exp_store(HALF, jbp, s_pre))
            out_chunk(pv_ps0, 0, 1)

            # --- attention pass 1 (i in [1024,2048)) ---
            pv_ps1 = ps_pv.tile([H + 1, HALF], f32, tag="pv", name="pv_ps")
            s_cur = attn_S(HALF, npre)
            p_cur = attn_exp(HALF, npre, s_cur)
            for jb in range(16):
                if jb < npre:
                    attn_pv(HALF, 16, pv_ps1, jb, pre_p[jb])
                    continue
                s_nxt = attn_S(HALF, jb + 1) if jb + 1 < 16 else None
                attn_pv(HALF, 16, pv_ps1, jb, p_cur)
                if s_nxt is not None:
                    p_cur = attn_exp(HALF, jb + 1, s_nxt)
                if jb == 11:
                    out_chunk(pv_ps1, HALF, 0)
                if jb == npre:
                    vtrans_unit((10, 11))
                elif jb == npre + 1:
                    vtrans_unit((12, 13))
                elif jb == npre + 2:
                    vtrans_unit((14, 15))
            out_chunk(pv_ps1, HALF, 1)

    _legalize_waits(nc)
    return nc


def build_in_maps(x, Wq, Wk, Wv):
    x = np.ascontiguousarray(np.asarray(x), dtype=np.float32)
    wqk_np = np.ascontiguousarray(
        np.concatenate([np.asarray(Wq), np.asarray(Wk)], axis=1), dtype=np.float32
    )
    wv_np = np.ascontiguousarray(np.asarray(Wv), dtype=np.float32)

    def ctile_pack(a, w):  # [512, w] -> [128, 4*w] with c-tiles side by side
        return a.reshape(4, 128, w).transpose(1, 0, 2).reshape(128, 4 * w)

    cw_np = np.ascontiguousarray(ctile_pack(wqk_np.astype(np.float16), 128))
    mask_np = np.triu(np.ones((128, 128), dtype=np.float16))
    ident_np = np.zeros((128, H), dtype=np.float16)
    ident_np[:H] = np.eye(H, dtype=np.float16)
    ones_np = np.ones((128, T // 128), dtype=np.float16)
    crest_np = np.ascontiguousarray(
        np.concatenate(
            [
                ctile_pack(wv_np.astype(np.float16), 64),
                mask_np,
                ones_np,
                ident_np,
            ],
            axis=1,
        )
    )

    maps = []
    for b in range(N_CORES):
        xt = x[b].astype(np.float16)  # [T, D]
        # plane tc: [128p, 4c, 512t'] with xt[512tc+t', 128c+p]
        planes = xt.reshape(4, 512, 4, 128).transpose(0, 3, 2, 1)
        m = {"cw": cw_np, "crest": crest_np}
        for tc in range(4):
            m[f"xt{tc}"] = np.ascontiguousarray(planes[tc].reshape(128, 2048))
        maps.append(m)
    return maps


def kernel(x, Wq, Wk, Wv):
    from concourse.bass_utils import run_bass_kernel_spmd

    if "nc" not in _cache:
        _cache["nc"] = _build()
    nc = _cache["nc"]

    in_maps = build_in_maps(x, Wq, Wk, Wv)
    res = run_bass_kernel_spmd(nc, in_maps, list(range(N_CORES))).results

    out = np.empty((B, T, H), dtype=np.float32)
    for b in range(N_CORES):
        strip = res[b]["out"]  # [H+1, T]
        out[b] = (strip[:H, :] / strip[H : H + 1, :]).T
    return out


if __name__ == "__main__":
    rng = np.random.default_rng(0)
    x = rng.standard_normal((B, T, D)).astype(np.float32)
    s = 1.0 / np.sqrt(D)
    Wq = (rng.standard_normal((D, H)) * s).astype(np.float32)
    Wk = (rng.standard_normal((D, H)) * s).astype(np.float32)
    Wv = (rng.standard_normal((D, H)) * s).astype(np.float32)
    out = kernel(x=x, Wq=Wq, Wk=Wk, Wv=Wv)
    print("out", out.shape, out.dtype, np.abs(out).max())
